# revision 38
# baseline (speedup 1.0000x reference)
"""DWAMFormer frame-merge block on 8 Trainium2 NeuronCores.

Math (per the reference):
  flat = windows of x: (B*Tw, C*MS) with feature order (c, m)
  y  = sigmoid(relu(flat @ w1) @ w2)
  att = softmax over the MS window positions within each channel group
  pooled = sum_m flat * att
  out = layernorm(pooled @ fc_w + fc_b)

Strategy: data-parallel over batch B (2 batches per core), weights
replicated. Both big matmuls run in fp8 e4m3 with DoubleRow perf mode
(2 contraction rows per PE cell -> 2x bf16 throughput). Power-of-2
scale factors keep all fp8 values inside TRN's +/-240 e4m3 range and
are folded into the ACT scale of the following activation.

Structure per core (R=1600 rows):
  Phase A (feature-major): hT = relu(w1.T @ flatT) with w1 stationary
  (fp8 DoubleRow tiles [128,2,128]) and flatT moving in two 800-row
  halves; relu output written as fp8 to a DRAM scratch tensor hTd.
  Phase B (row-major): per 128-row tile, y = hT_tile.T @ w2 with the
  row-tile of hT as the stationary operand (LDWEIGHTS amortized over
  5 moving w2 tiles) and w2 fully SBUF-resident as the moving operand.
  The output lands row-major, so sigmoid -> exp -> softmax-over-m ->
  pooling against a bf16 row-major copy of x -> fc (f32r) -> layernorm
  all proceed without extra transposes (only pooled needs a 128x128 PE
  transpose before the fc matmul).

Feature permutation trick: the reference's window features are ordered
(c, m) = c*MS + m, which would need a strided on-chip gather. We
instead use the order (m, c) = m*C + c, under which `flat` is exactly
x.reshape(rows, MS*C) -- contiguous. w1 rows / w2 cols are permuted to
match on the host (pure relabeling of the MLP's in/out features).
"""

import numpy as np
import ml_dtypes

import concourse.bass as bass
import concourse.mybir as mybir
import concourse.tile as tile
from concourse import bacc
from concourse import bass_utils

# Problem sizes (fixed by the task).
B, T, C = 16, 4000, 512
MS = 5
TW = T // MS              # 800 windows per batch
D = C * MS                # 2560 window features
DH = 2 * D                # 5120 hidden features
N_CORES = 8
BPC = B // N_CORES        # 2 batches per core
R = BPC * TW              # 1600 rows per core
P = 128
K1 = D // P               # 20 input-feature chunks
KP1 = K1 // 2             # 10 DoubleRow K-pairs for matmul1
KH = DH // P              # 40 hidden chunks
KP2 = KH // 2             # 20 DoubleRow K-pairs for matmul2
MT = DH // P              # 40 output tiles of matmul1
CG = C // P               # 4 channel groups
NRT = (R + P - 1) // P    # 13 row-tiles in phase B (12x128 + 64)
EPS = 1e-5

# fp8 scale factors (powers of 2; TRN e4m3 max normal is +/-240).
SX = 16.0                 # x absmax ~5.4  -> 87
S1 = 4096.0               # w1 absmax ~0.028 -> 115
SH = 32.0                 # h absmax ~5 -> 160
S2 = 4096.0               # w2 absmax ~0.028 -> 115
RELU_SCALE = SH / (SX * S1)      # 2^-11
SIG_SCALE = 1.0 / (SH * S2)      # 2^-17

F32 = mybir.dt.float32
F32R = mybir.dt.float32r
BF16 = mybir.dt.bfloat16
F8 = mybir.dt.float8e4
AF = mybir.ActivationFunctionType
ALU = mybir.AluOpType
DR = mybir.MatmulPerfMode.DoubleRow

F8NP = mybir.dt.np(F8)
BF16NP = ml_dtypes.bfloat16

# Tunables (experiments override before _build()).
CFG = {
    "reps": 1,
    "half_rows": 1600,    # rows per phase-A x half (flatT residency)
    "rb": 400,            # phase-A moving block (<=512 for one PSUM bank)
    "w1_bufs": 2,
    "hs_bufs": 4,
    "hrt_bufs": 2,
    "xrm_bufs": 2,
    "sig_bufs": 2,
    "skip_a": 0,
    "skip_b": 0,
    "ubench": 0,   # PE-only LDW+MM DoubleRow stream (pair-cost microbench)
}


def _bcast_ap(src: bass.AP, parts: int) -> bass.AP:
    """Partition-broadcast a 1-D DRAM AP for a replicating DMA."""
    return bass.AP(tensor=src.tensor, offset=src.offset, ap=[[0, parts]] + list(src.ap))


def _emit(tc, xq, xrm, w1r, w2r, fcw, fcb, lng, lnb, ident, out):
    nc = tc.nc
    import contextlib
    HR = CFG["half_rows"]
    NH = R // HR                      # phase-A halves
    RB = CFG["rb"]                    # moving block rows
    NB = HR // RB                     # moving blocks per half
    ctx = contextlib.ExitStack()
    with ctx:
        singles = ctx.enter_context(tc.tile_pool(name="singles", bufs=1))
        f2pool = ctx.enter_context(tc.tile_pool(name="f2pool", bufs=1))
        w1pool = ctx.enter_context(tc.tile_pool(name="w1pool", bufs=CFG["w1_bufs"]))
        hspool = ctx.enter_context(tc.tile_pool(name="hspool", bufs=CFG["hs_bufs"]))
        hrtpool = ctx.enter_context(tc.tile_pool(name="hrtpool", bufs=CFG["hrt_bufs"]))
        xrmpool = ctx.enter_context(tc.tile_pool(name="xrmpool", bufs=CFG["xrm_bufs"]))
        spool = ctx.enter_context(tc.tile_pool(name="spool", bufs=1))
        epool = ctx.enter_context(tc.tile_pool(name="epool", bufs=CFG["sig_bufs"]))
        dpool = ctx.enter_context(tc.tile_pool(name="dpool", bufs=1))
        ppool = ctx.enter_context(tc.tile_pool(name="ppool", bufs=2))
        ptpool = ctx.enter_context(tc.tile_pool(name="ptpool", bufs=1))
        cpool = ctx.enter_context(tc.tile_pool(name="cpool", bufs=2))
        drampool = ctx.enter_context(tc.tile_pool(name="drampool", bufs=1, space="DRAM"))

        # --- constants / resident tensors ---
        w2sb = singles.tile([P, KP2, MS, 2, C], F8)
        nc.sync.dma_start(out=w2sb, in_=w2r)
        fcw_sb = singles.tile([P, CG, C], F32R)
        nc.sync.dma_start(out=fcw_sb, in_=fcw.rearrange("(ko p) n -> p ko n", p=P))
        fcb_sb = singles.tile([P, C], BF16)
        nc.gpsimd.dma_start(out=fcb_sb, in_=_bcast_ap(fcb, P))
        lng_sb = singles.tile([P, C], BF16)
        nc.gpsimd.dma_start(out=lng_sb, in_=_bcast_ap(lng, P))
        lnb_sb = singles.tile([P, C], BF16)
        nc.gpsimd.dma_start(out=lnb_sb, in_=_bcast_ap(lnb, P))
        eps_sb = singles.tile([P, 1], F32)
        nc.vector.memset(eps_sb, EPS)
        ident_sb = singles.tile([P, P], F32)
        nc.sync.dma_start(out=ident_sb, in_=ident)

        hTd = drampool.tile([P, KH, R], F8)

        if CFG["ubench"]:
            # 2000 LDW+MM DoubleRow pairs, N=512, minimal cross-engine deps.
            f2u = f2pool.tile([P, K1, 1024], F8, tag="f2")
            nc.sync.dma_start(out=f2u, in_=xq[:, :, 0:1024].rearrange("k p r -> p k r"))
            w1u = w1pool.tile([P, KP1, 2, P], F8, tag="w1t")
            nc.sync.dma_start(out=w1u, in_=w1r[0])
            with tc.tile_pool(name="ps_u", bufs=8, space="PSUM") as ps_u:
                for rep in range(CFG["reps"]):
                    for g in range(100):
                        psu = ps_u.tile([P, C], F32, tag="u", name=f"psu_{rep}_{g}")
                        for i in range(20):
                            nc.tensor.matmul(
                                psu,
                                w1u[:, i % KP1],
                                f2u[:, 2 * (i % KP1):2 * (i % KP1) + 2, 0:C],
                                start=(i == 0), stop=(i == 19),
                                perf_mode=DR,
                            )
                        h = cpool.tile([P, C], F32, tag="h", name=f"uh_{rep}_{g}")
                        nc.vector.tensor_copy(out=h, in_=psu)
                        if g == 99:
                            nc.gpsimd.dma_start(out=out[0:P, :], in_=h)
            return

        for rep in range(CFG["reps"]):
            # =============== Phase A: hT = relu(flat @ w1), fp8 ===============
            with tc.tile_pool(name="ps_a", bufs=8, space="PSUM") as ps_a:
                for half in (range(NH) if not CFG["skip_a"] else []):
                    f2 = f2pool.tile([P, K1, HR], F8, tag="f2")
                    nc.sync.dma_start(
                        out=f2,
                        in_=xq[:, :, half * HR:(half + 1) * HR].rearrange(
                            "k p r -> p k r"
                        ),
                    )
                    for mt in range(MT):
                        w1t = w1pool.tile([P, KP1, 2, P], F8, tag="w1t")
                        nc.sync.dma_start(out=w1t, in_=w1r[mt])
                        pss = [
                            ps_a.tile([P, RB], F32, tag="acc", name=f"psa_{half}_{mt}_{b}")
                            for b in range(NB)
                        ]
                        for kp in range(KP1):
                            for b in range(NB):
                                nc.tensor.matmul(
                                    pss[b],
                                    w1t[:, kp],
                                    f2[:, 2 * kp:2 * kp + 2, b * RB:(b + 1) * RB],
                                    start=(kp == 0), stop=(kp == KP1 - 1),
                                    perf_mode=DR,
                                )
                        for b in range(NB):
                            hs = hspool.tile([P, RB], F8, tag="hs")
                            nc.scalar.activation(
                                out=hs, in_=pss[b], func=AF.Relu, scale=RELU_SCALE
                            )
                            r0 = half * HR + b * RB
                            nc.gpsimd.dma_start(
                                out=hTd[:, mt, r0:r0 + RB], in_=hs
                            )

            # ====== Phase B: y=sigmoid(h @ w2); softmax over m; pool; fc; LN ======
            ctx_b = contextlib.ExitStack()
            ps_y = ctx_b.enter_context(
                tc.tile_pool(name=f"ps_y{rep}", bufs=MS, space="PSUM")
            )
            ps_t = ctx_b.enter_context(
                tc.tile_pool(name=f"ps_t{rep}", bufs=2, space="PSUM")
            )
            ps_c = ctx_b.enter_context(
                tc.tile_pool(name=f"ps_c{rep}", bufs=1, space="PSUM")
            )

            def _emit_tail(rt, r0, rsz, pooled):
                # pooledT via PE transpose, then fc matmul (f32r) + layernorm
                pooledT = ptpool.tile([P, CG, P], F32R, tag="pooledT")
                pt_ps = ps_t.tile([P, CG, P], F32, tag="pt", name=f"pt_{rt}")
                for kc in range(CG):
                    nc.tensor.transpose(
                        pt_ps[:, kc, :rsz],
                        pooled[:rsz, kc * P:(kc + 1) * P],
                        ident_sb[:rsz, :rsz],
                    )
                nc.vector.tensor_copy(
                    out=pooledT[:, :, :rsz], in_=pt_ps[:, :, :rsz]
                )
                pso = ps_c.tile([P, C], F32, tag="pso", name=f"pso_{rt}")
                for kc in range(CG):
                    nc.tensor.matmul(
                        pso[:rsz],
                        pooledT[:, kc, :rsz],
                        fcw_sb[:, kc],
                        start=(kc == 0), stop=(kc == CG - 1),
                    )
                h = cpool.tile([P, C], F32, tag="h", name=f"h_{rt}")
                nc.vector.tensor_add(h[:rsz], pso[:rsz], fcb_sb[:rsz])
                stats = cpool.tile(
                    [P, nc.vector.BN_STATS_DIM], F32, tag="st", name=f"st_{rt}"
                )
                nc.vector.bn_stats(out=stats[:rsz], in_=h[:rsz])
                mv = cpool.tile(
                    [P, nc.vector.BN_AGGR_DIM], F32, tag="mv", name=f"mv_{rt}"
                )
                nc.vector.bn_aggr(out=mv[:rsz], in_=stats[:rsz])
                # rstd = exp(-0.5*ln(var+eps)) -- ln/exp share the act table.
                nc.scalar.activation(
                    out=mv[:rsz, 1:2], in_=mv[:rsz, 1:2], func=AF.Ln,
                    bias=eps_sb[:rsz],
                )
                nc.scalar.activation(
                    out=mv[:rsz, 1:2], in_=mv[:rsz, 1:2], func=AF.Exp,
                    scale=-0.5,
                )
                nc.vector.tensor_scalar(
                    h[:rsz], h[:rsz], mv[:rsz, 0:1], mv[:rsz, 1:2],
                    ALU.subtract, ALU.mult,
                )
                nc.vector.tensor_mul(h[:rsz], h[:rsz], lng_sb[:rsz])
                nc.vector.tensor_add(h[:rsz], h[:rsz], lnb_sb[:rsz])
                nc.gpsimd.dma_start(out=out[r0:r0 + rsz, :], in_=h[:rsz])

            pend = None
            for rt in (range(NRT) if not CFG["skip_b"] else []):
                r0 = rt * P
                rsz = min(P, R - r0)
                hrt = hrtpool.tile([P, KH, P], F8, tag="hrt")
                nc.sync.dma_start(out=hrt[:, :, :rsz], in_=hTd[:, :, r0:r0 + rsz])
                xrmt = xrmpool.tile([P, MS, C], BF16, tag="xrmt")
                nc.gpsimd.dma_start(
                    out=xrmt[:rsz], in_=xrm[r0:r0 + rsz].rearrange(
                        "r (m c) -> r m c", m=MS
                    )
                )

                psy = [
                    ps_y.tile([P, C], F32, tag="psy", name=f"psy_{rt}_{m}")
                    for m in range(MS)
                ]
                for kp in range(KP2):
                    for m in range(MS):
                        nc.tensor.matmul(
                            psy[m][:rsz],
                            hrt[:, 2 * kp:2 * kp + 2, :rsz],
                            w2sb[:, kp, m],
                            start=(kp == 0), stop=(kp == KP2 - 1),
                            perf_mode=DR,
                        )
                # e = exp(sigmoid(y)) computed with the exp table only, so the
                # whole kernel needs a single activation-table load:
                #   u = exp(-y); sig = 1/(1+u); e = exp(sig).
                # sig stays f32 (reciprocal_approx_fast needs fp32 bits); e is
                # bf16 so the softmax/pool elementwise ops hit the DVE 2x/4x
                # all-16-bit fast paths.
                sig = spool.tile([P, MS, C], F32, tag="sig")
                for m in range(MS):
                    nc.scalar.activation(
                        out=sig[:rsz, m], in_=psy[m][:rsz], func=AF.Exp,
                        scale=-SIG_SCALE,
                    )
                nc.vector.tensor_scalar_add(sig[:rsz], sig[:rsz], 1.0)
                nc.vector.reciprocal_approx_fast(out=sig[:rsz], in_=sig[:rsz])
                ebf = epool.tile([P, MS, C], BF16, tag="ebf")
                nc.scalar.activation(out=ebf[:rsz], in_=sig[:rsz], func=AF.Exp)
                # softmax-over-m denominator (tree of contiguous bf16 adds)
                s01 = dpool.tile([P, C], BF16, tag="s01")
                s23 = dpool.tile([P, C], BF16, tag="s23")
                with nc.allow_low_precision(reason="softmax sums fine in bf16"):
                    nc.vector.tensor_add(s01[:rsz], ebf[:rsz, 0], ebf[:rsz, 1])
                    nc.vector.tensor_add(s23[:rsz], ebf[:rsz, 2], ebf[:rsz, 3])
                    nc.vector.tensor_add(s01[:rsz], s01[:rsz], s23[:rsz])
                s = dpool.tile([P, C], F32, tag="s")
                nc.vector.tensor_add(s[:rsz], s01[:rsz], ebf[:rsz, 4])
                rcp = dpool.tile([P, C], F32, tag="rcp")
                nc.vector.reciprocal_approx_fast(out=rcp[:rsz], in_=s[:rsz])
                # numerator: e * x elementwise (one bf16 op), then tree-sum
                with nc.allow_low_precision(reason="pool products fine in bf16"):
                    nc.vector.tensor_mul(ebf[:rsz], ebf[:rsz], xrmt[:rsz])
                    nc.vector.tensor_add(s01[:rsz], ebf[:rsz, 0], ebf[:rsz, 1])
                    nc.vector.tensor_add(s23[:rsz], ebf[:rsz, 2], ebf[:rsz, 3])
                    nc.vector.tensor_add(s01[:rsz], s01[:rsz], s23[:rsz])
                acc = dpool.tile([P, C], F32, tag="pacc")
                nc.vector.tensor_add(acc[:rsz], s01[:rsz], ebf[:rsz, 4])
                pooled = ppool.tile([P, C], F32, tag="pooled")
                nc.vector.tensor_mul(pooled[:rsz], acc[:rsz], rcp[:rsz])

                # Tail (transpose/fc/LN/store) lags one row-tile so the PE
                # stream never waits on the DVE pooling chain.
                if pend is not None:
                    _emit_tail(*pend)
                pend = (rt, r0, rsz, pooled)
            if pend is not None:
                _emit_tail(*pend)
            ctx_b.close()


_ONE_TABLE = "natural_log_exp_and_others"


def _patched_tables(orig):
    """Steer the act-table-load chooser to the single set that covers every
    activation function this kernel uses (exp, ln, relu, copy, identity), so
    the whole kernel needs one table load instead of thrashing per row-tile.
    Only set MEMBERSHIP seen by the chooser changes; set indices (the runtime
    act_func_set_id mapping) are untouched.
    """
    ours = {AF.Exp, AF.Ln, AF.Relu, AF.Copy, AF.Identity}

    def patched(arch):
        tables = orig(arch)
        return {
            name: (funcs if name == _ONE_TABLE else funcs - ours)
            for name, funcs in tables.items()
        }

    return patched


def _build():
    orig = bacc.get_activation_tables
    bacc.get_activation_tables = _patched_tables(orig)
    try:
        return _build_inner()
    finally:
        bacc.get_activation_tables = orig


def _build_inner():
    nc = bacc.Bacc(
        "TRN2", target_bir_lowering=False, debug=False, num_devices=N_CORES
    )
    xq = nc.dram_tensor("xq", [K1, P, R], F8, kind="ExternalInput").ap()
    xrm = nc.dram_tensor("xrm", [R, D], BF16, kind="ExternalInput").ap()
    w1r = nc.dram_tensor("w1r", [MT, P, KP1, 2, P], F8, kind="ExternalInput").ap()
    w2r = nc.dram_tensor("w2r", [P, KP2, MS, 2, C], F8, kind="ExternalInput").ap()
    fcw = nc.dram_tensor("fcw", [C, C], F32R, kind="ExternalInput").ap()
    fcb = nc.dram_tensor("fcb", [C], F32, kind="ExternalInput").ap()
    lng = nc.dram_tensor("lng", [C], F32, kind="ExternalInput").ap()
    lnb = nc.dram_tensor("lnb", [C], F32, kind="ExternalInput").ap()
    ident = nc.dram_tensor("ident", [P, P], F32, kind="ExternalInput").ap()
    out = nc.dram_tensor("out", [R, C], F32, kind="ExternalOutput").ap()
    with tile.TileContext(nc) as tc:
        _emit(tc, xq, xrm, w1r, w2r, fcw, fcb, lng, lnb, ident, out)
    nc.compile()
    return nc


_STATE: dict = {}


def _fp8(a: np.ndarray, scale: float) -> np.ndarray:
    return np.clip(a * scale, -240.0, 240.0).astype(F8NP)


def _prep_weights(w1, w2):
    w1 = np.asarray(w1, dtype=np.float32)
    w2 = np.asarray(w2, dtype=np.float32)
    # w1 rows from (c, m) = c*MS + m order to f = m*C + c order.
    w1p = w1.reshape(C, MS, DH).transpose(1, 0, 2).reshape(D, DH)
    # Tiles [mt, p, kp, j, mcol]: contraction k = kp*256 + j*128 + p.
    w1q = _fp8(w1p, S1)
    w1r = np.ascontiguousarray(
        w1q.reshape(KP1, 2, P, MT, P).transpose(3, 2, 0, 1, 4)
    )
    # w2 cols from (c, m) = c*MS + m order to f' = m*C + c order.
    w2p = w2.reshape(DH, C, MS).transpose(0, 2, 1).reshape(DH, D)
    # Tiles [p, kp, m, j, c]: contraction k = kp*256 + j*128 + p.
    w2q = _fp8(w2p, S2)
    w2r = np.ascontiguousarray(
        w2q.reshape(KP2, 2, P, MS, C).transpose(2, 0, 3, 1, 4)
    )
    return w1r, w2r


def _fingerprint(inputs):
    parts = []
    for k in ("w1", "w2", "fc_w", "fc_b", "ln_g", "ln_b"):
        a = np.asarray(inputs[k])
        flat = a.reshape(-1)
        parts.append((a.shape, flat[:: max(1, flat.size // 256)].tobytes()))
    return hash(repr(parts))


def make_in_maps(inputs) -> list:
    x = np.asarray(inputs["x"], dtype=np.float32)
    fp = _fingerprint(inputs)
    if _STATE.get("w_fp") != fp:
        _STATE["w"] = _prep_weights(inputs["w1"], inputs["w2"])
        _STATE["w_fp"] = fp
        _STATE.pop("static_fp", None)
    w1r, w2r = _STATE["w"]
    fcw = np.asarray(inputs["fc_w"], dtype=np.float32)
    fcb = np.asarray(inputs["fc_b"], dtype=np.float32)
    lng = np.asarray(inputs["ln_g"], dtype=np.float32)
    lnb = np.asarray(inputs["ln_b"], dtype=np.float32)
    ident = np.eye(P, dtype=np.float32)
    in_maps = []
    for c in range(N_CORES):
        xc = x[c * BPC:(c + 1) * BPC].reshape(R, D)
        xq = np.ascontiguousarray(_fp8(xc, SX).T.reshape(K1, P, R))
        xrm = xc.astype(BF16NP)
        in_maps.append({
            "xq": xq, "xrm": xrm, "w1r": w1r, "w2r": w2r, "fcw": fcw,
            "fcb": fcb, "lng": lng, "lnb": lnb, "ident": ident,
        })
    return in_maps


def kernel(**inputs) -> np.ndarray:
    if "nc" not in _STATE:
        _STATE["nc"] = _build()
    in_maps = make_in_maps(inputs)
    from concourse._compat import axon_active
    if not axon_active():
        res = bass_utils.run_bass_kernel_spmd(
            _STATE["nc"], in_maps, core_ids=list(range(N_CORES)), trace=False
        )
        outs = [res.results[c]["out"].reshape(BPC, TW, C) for c in range(N_CORES)]
        return np.concatenate(outs, axis=0)
    if "runner" not in _STATE:
        _STATE["runner"] = _Runner(_STATE["nc"], N_CORES)
    if _STATE.get("static_fp") != _STATE.get("w_fp"):
        _STATE["runner"].put_static(
            in_maps, {"w1r", "w2r", "fcw", "fcb", "lng", "lnb", "ident"}
        )
        _STATE["static_fp"] = _STATE.get("w_fp")
    res = _STATE["runner"].run(in_maps)
    outs = [res[c]["out"].reshape(BPC, TW, C) for c in range(N_CORES)]
    return np.concatenate(outs, axis=0)


class _Runner:
    """Persistent PJRT SPMD executor (axon path): keeps the jitted NEFF and
    device-resident replicated inputs alive across calls."""

    def __init__(self, nc, n_cores):
        import jax
        from jax.sharding import Mesh, PartitionSpec
        from jax.experimental.shard_map import shard_map
        from concourse import bass2jax
        bass2jax.install_neuronx_cc_hook()
        self.jax = jax
        self.n_cores = n_cores
        partition_name = (
            nc.partition_id_tensor.name if nc.partition_id_tensor else None
        )
        in_names, out_names, out_avals, zero_outs = [], [], [], []
        for alloc in nc.m.functions[0].allocations:
            if not isinstance(alloc, mybir.MemoryLocationSet):
                continue
            name = alloc.memorylocations[0].name
            if alloc.kind == "ExternalInput":
                if name != partition_name:
                    in_names.append(name)
            elif alloc.kind == "ExternalOutput":
                shape = tuple(alloc.tensor_shape)
                dtype = mybir.dt.np(alloc.dtype)
                out_names.append(name)
                out_avals.append(jax.core.ShapedArray(shape, dtype))
                zero_outs.append(np.zeros(shape, dtype))
        self.in_names, self.out_names = in_names, out_names
        self.out_avals, self.zero_outs = out_avals, zero_outs
        n_params, n_outs = len(in_names), len(out_avals)
        all_in_names = in_names + out_names
        if partition_name is not None:
            all_in_names.append(partition_name)

        import jax.numpy as jnp

        def _body(*args):
            operands = list(args)
            if partition_name is not None:
                operands.append(bass2jax.partition_id_tensor())
            return tuple(bass2jax._bass_exec_p.bind(
                *operands,
                out_avals=tuple(out_avals),
                in_names=tuple(all_in_names),
                out_names=tuple(out_names),
                lowering_input_output_aliases=(),
                sim_require_finite=True,
                sim_require_nnan=True,
                nc=nc,
            ))

        devices = jax.devices()[:n_cores]
        self.mesh = Mesh(np.asarray(devices), ("core",))
        in_specs = (PartitionSpec("core"),) * (n_params + n_outs)
        out_specs = (PartitionSpec("core"),) * n_outs
        self.sharded = jax.jit(
            shard_map(_body, mesh=self.mesh, in_specs=in_specs,
                      out_specs=out_specs, check_rep=False),
            donate_argnums=tuple(range(n_params, n_params + n_outs)),
            keep_unused=True,
        )
        # Device-side zeros maker: output operand buffers never cross the
        # tunnel (they are donated to the kernel each call).
        from jax.sharding import NamedSharding
        zsh = NamedSharding(self.mesh, PartitionSpec("core"))
        self._mk_zeros = jax.jit(
            lambda: tuple(
                jnp.zeros((n_cores * z.shape[0], *z.shape[1:]), z.dtype)
                for z in zero_outs
            ),
            out_shardings=(zsh,) * n_outs,
        )
        self._static = {}

    def _concat(self, in_maps, name):
        return np.concatenate([np.asarray(m[name]) for m in in_maps], axis=0)

    def put_static(self, in_maps, names):
        from jax.sharding import NamedSharding, PartitionSpec
        sh = NamedSharding(self.mesh, PartitionSpec("core"))
        for name in names:
            if name in self.in_names:
                self._static[name] = self.jax.device_put(
                    self._concat(in_maps, name), sh
                )

    def run(self, in_maps, device_out=False):
        args = [
            self._static[name] if name in self._static
            else self._concat(in_maps, name)
            for name in self.in_names
        ]
        out_arrs = self.sharded(*args, *self._mk_zeros())
        if device_out:
            return out_arrs
        return [
            {
                name: np.asarray(out_arrs[i]).reshape(
                    self.n_cores, *self.out_avals[i].shape
                )[c]
                for i, name in enumerate(self.out_names)
            }
            for c in range(self.n_cores)
        ]


if __name__ == "__main__":
    import time
    t0 = time.time()
    _build()
    print(f"build+compile OK in {time.time() - t0:.1f}s")
